# revision 21
# baseline (speedup 1.0000x reference)
"""DenoiseGAT Trainium2 kernel: 8-core data-parallel over polygons (cycle graphs).

v2.1: layer-major software pipeline. Chunks of 2048 nodes (32 polygons)
stream through each GAT layer in waves of 4; per layer-phase, chunk c's
attention combine is emitted while chunk c+1's matmul/softmax stage runs,
so every engine sequencer sees a dense in-order stream. Activations h^T
(features x nodes, bf16) in [128, 2, n] half tiles. Scores come straight
from h via host-folded W@a_blk; softmax runs in a (poly, head)-packed
partition layout [128, 3, 64]; alphas return to feature rows via a
DRAM-staged broadcast DMA pair; the neighbor combine is +-1 shifted DVE
tensor ops inside 64-node polygons.
"""

import numpy as np
import ml_dtypes
from contextlib import ExitStack

import concourse.bass as bass
import concourse.tile as tile
import concourse.tile_utils as tile_utils
from concourse import bacc, mybir
from concourse.bass_utils import run_bass_kernel_spmd

tile_utils.max_sbuf_usage = 208 * 1024

F32 = mybir.dt.float32
BF16 = mybir.dt.bfloat16
ALU = mybir.AluOpType
ACTF = mybir.ActivationFunctionType

NCORES = 8
B, V = 2048, 64
HID, TDIM = 256, 128
R = 4                       # heads in layers 0-2
BC = B // NCORES            # 256 polygons / core
N = BC * V                  # 16384 nodes / core
CCH = 2048                  # chunk nodes = 32 polys
PCH = CCH // V              # 32 polys / chunk
NT = 512                    # matmul span (one PSUM bank col-width)
NCH = N // CCH              # 8 chunks
SUB = 1024                  # combine sub-chunk
WAVE = 4                    # chunks per wave


def _ablk(asrc, atgt):
    NH, FO = asrc.shape
    out = np.zeros((NH * FO, 2 * NH), np.float32)
    for h in range(NH):
        out[h * FO:(h + 1) * FO, h] = asrc[h]
        out[h * FO:(h + 1) * FO, NH + h] = atgt[h]
    return out


def _bf(a):
    return np.ascontiguousarray(np.asarray(a, np.float32).astype(ml_dtypes.bfloat16))


def _f32(a):
    return np.ascontiguousarray(np.asarray(a, np.float32))


def half3(a):
    """(256, X) host -> (128, 2, X) so tile[:, j, :] == rows 128j:128j+128."""
    a = np.asarray(a)
    return np.ascontiguousarray(a.reshape(2, 128, a.shape[1]).transpose(1, 0, 2))


def build(weights):
    nc = bacc.Bacc("TRN2", target_bir_lowering=False, debug=False,
                   enable_asserts=False, num_devices=NCORES)
    w = weights

    def inl(name, arr):
        return nc.inline_tensor(np.ascontiguousarray(arr), name=name).ap()

    half = TDIM // 2
    freqs = np.exp(-np.log(10000.0) * np.arange(half, dtype=np.float32) / (half - 1))
    fr2 = np.stack([np.concatenate([freqs, freqs]),
                    np.concatenate([np.zeros(half, np.float32),
                                    np.full(half, np.pi / 2, np.float32)])])

    W0 = _f32(w["W0"]); sk0 = _f32(w["skip0"]); ab0 = _ablk(_f32(w["asrc0"]), _f32(w["atgt0"]))
    c_fr2 = inl("fr2", fr2.astype(np.float32))
    c_tW = inl("tW", _f32(w["tW"]))
    c_tb = inl("tb", _f32(w["tb"]).reshape(-1, 1))
    c_Wsum0t = inl("Wsum0t", W0[6:] + sk0[6:])            # (128, 256) f32
    c_W0abt = inl("W0abt", W0[6:] @ ab0)                  # (128, 8) f32
    c_W0f = inl("W0f", _bf(np.concatenate([W0[:6], sk0[:6]], 1)))   # (6, 512)
    c_W0ab6 = inl("W0ab6", _bf(W0[:6] @ ab0))             # (6, 8)
    c_b0 = inl("b0c", half3(_f32(w["b0"]).reshape(-1, 1)))
    LW, LAB, LB = {}, {}, {}
    for i in (1, 2):
        Wi = _f32(w[f"W{i}"])
        abi = _ablk(_f32(w[f"asrc{i}"]), _f32(w[f"atgt{i}"]))
        LW[i] = inl(f"W{i}f", half3(_bf(np.concatenate([Wi, _f32(w[f"skip{i}"])], 1))))
        LAB[i] = inl(f"ab{i}f", half3(_bf(Wi @ abi)))     # (128, 2, 8)
        LB[i] = inl(f"b{i}c", half3(_f32(w[f"b{i}"]).reshape(-1, 1)))
    W3 = _f32(w["W3"]); ab3 = _ablk(_f32(w["asrc3"]), _f32(w["atgt3"]))
    c_W3 = inl("W3f", half3(_bf(W3)))
    c_Wab3 = inl("Wab3", half3(_bf(W3 @ ab3)))            # (128, 2, 2)
    c_b3 = inl("b3c", half3(_f32(w["b3"]).reshape(-1, 1)))
    c_h1W = inl("h1Wf", half3(_bf(_f32(w["h1W"]))))
    c_h1b = inl("h1bc", half3(_f32(w["h1b"]).reshape(-1, 1)))
    c_h2W = inl("h2Wf", half3(_bf(_f32(w["h2W"]))))
    c_h2b = inl("h2bc", _f32(w["h2b"]).reshape(-1, 1))

    h6 = nc.dram_tensor("h6", [6, N], BF16, kind="ExternalInput").ap()
    tp = nc.dram_tensor("tp", [2, BC], F32, kind="ExternalInput").ap()
    yT = nc.dram_tensor("yT", [2, N], F32, kind="ExternalOutput").ap()

    with tile.TileContext(nc) as tc, ExitStack() as ctx:
        WP = ctx.enter_context(tc.tile_pool(name="wts", bufs=1))
        P = ctx.enter_context(tc.tile_pool(name="pers", bufs=1))
        HP = ctx.enter_context(tc.tile_pool(name="hp", bufs=10))
        CH = ctx.enter_context(tc.tile_pool(name="ch", bufs=2))
        SM = ctx.enter_context(tc.tile_pool(name="sm", bufs=3))
        DR = ctx.enter_context(tc.tile_pool(name="dram", bufs=8, space="DRAM"))
        DR1 = ctx.enter_context(tc.tile_pool(name="dram1", bufs=1, space="DRAM"))
        PP = ctx.enter_context(tc.tile_pool(name="pp", bufs=2, space="PSUM"))
        PSK = ctx.enter_context(tc.tile_pool(name="psk", bufs=1, space="PSUM"))
        SCP = ctx.enter_context(tc.tile_pool(name="scp", bufs=2, space="PSUM"))

        TT = nc.vector.tensor_tensor
        PT = nc.gpsimd.tensor_tensor
        STT = nc.vector.scalar_tensor_tensor
        TSP = nc.vector.tensor_scalar

        def load(c_ap, tag):
            t = WP.tile(list(c_ap.shape), c_ap.dtype, tag=tag)
            nc.sync.dma_start(t[:], c_ap)
            return t

        # ---- temb preamble (once) ----
        t_fr2 = load(c_fr2, "fr2")
        t_tp = load(tp, "tp")
        ps_te = PP.tile([TDIM, BC], F32, tag="pp", name="ps_te")
        nc.tensor.matmul(ps_te[:], t_fr2[:], t_tp[:], start=True, stop=True)
        te_m = SM.tile([TDIM, BC], F32, tag="te", name="te_m")
        te_q = SM.tile([TDIM, BC], mybir.dt.int32, tag="teq")
        TSP(te_q[:], ps_te[:], float(1.0 / (2 * np.pi)), None, op0=ALU.mult)
        te_qf = SM.tile([TDIM, BC], F32, tag="te", name="te_qf")
        nc.vector.tensor_copy(te_qf[:], te_q[:])
        STT(te_m[:], te_qf[:], float(-2 * np.pi), ps_te[:], op0=ALU.mult, op1=ALU.add)
        te_s = SM.tile([TDIM, BC], F32, tag="te", name="te_s")
        nc.scalar.activation(te_s[:], te_m[:], ACTF.Sin)
        t_tW = load(c_tW, "tW")
        t_tb = load(c_tb, "tb")
        ps_tm = PP.tile([TDIM, BC], F32, tag="pp", name="ps_tm")
        nc.tensor.matmul(ps_tm[:], t_tW[:], te_s[:], start=True, stop=True)
        tembT = P.tile([TDIM, BC], F32, tag="tembT")
        nc.scalar.activation(tembT[:], ps_tm[:], ACTF.Silu, bias=t_tb[:])

        # G0 = (W0[6:]+skip0[6:]) @ temb : per-graph proj+skip contribution
        t_Ws0 = load(c_Wsum0t, "Ws0")
        G0b = P.tile([128, 2, BC], BF16, tag="G0b")
        for m in range(2):
            ps_g = PP.tile([128, BC], F32, tag="pp", name=f"ps_g{m}")
            nc.tensor.matmul(ps_g[:], t_Ws0[:, m * 128:(m + 1) * 128], tembT[:],
                             start=True, stop=True)
            nc.scalar.activation(G0b[:, m, :], ps_g[:], ACTF.Copy)
        # s0 = (W0[6:]@ab0)^T temb : per-graph score additive, (poly,head)-packed
        t_W0abt = load(c_W0abt, "W0abt")
        ps_sg = SCP.tile([8, BC], F32, tag="scp", name="ps_sg")
        nc.tensor.matmul(ps_sg[:], t_W0abt[:], tembT[:], start=True, stop=True)
        s0s = SM.tile([8, BC], BF16, tag="s0s")
        nc.scalar.activation(s0s[:], ps_sg[:], ACTF.Copy)
        s0_dram = DR1.tile([8, BC], BF16, tag="s0d")
        nc.sync.dma_start(s0_dram[:], s0s[:])
        s0a = P.tile([128, 2, NCH], BF16, tag="s0a")
        s0v = s0_dram[:].rearrange("(t r) (c p) -> r t p c", t=2, p=PCH)
        for r in range(R):
            for tt_ in range(2):
                nc.sync.dma_start(s0a[PCH * r:PCH * r + PCH, tt_, :], s0v[r, tt_])

        # ---- persistent weights ----
        t_W = {0: load(c_W0f, "W0f"), 1: load(LW[1], "W1"),
               2: load(LW[2], "W2"), 3: load(c_W3, "W3")}
        t_ab = {0: load(c_W0ab6, "W0ab6"), 1: load(LAB[1], "ab1"),
                2: load(LAB[2], "ab2"), 3: load(c_Wab3, "ab3")}
        t_b = {0: load(c_b0, "b0"), 1: load(LB[1], "b1"),
               2: load(LB[2], "b2"), 3: load(c_b3, "b3")}
        t_h1W = load(c_h1W, "h1W")
        t_h1b = load(c_h1b, "h1b")
        t_h2W = load(c_h2W, "h2W")
        t_h2b = load(c_h2b, "h2b")

        def stageA(li, ch, h_in):
            """Matmuls + evac + scores + softmax + alpha DMAs for one chunk.
            Returns state for stageB (the combine)."""
            first = li == 0
            last = li == 3
            Rl = 1 if last else R
            SR = 2 * Rl
            kt = 1 if first else 2
            tw, tab, tb = t_W[li], t_ab[li], t_b[li]

            if first:
                h_in = CH.tile([6, CCH], BF16, tag="h0")
                nc.sync.dma_start(h_in[:], h6[:, ch * CCH:(ch + 1) * CCH])
            projc = CH.tile([128, 2, CCH], BF16, tag="projc")
            acc = CH.tile([128, 2, CCH], BF16, tag="acc")
            scT = SM.tile([128, NT], BF16, tag="scT")

            def lhs_w(k, m):
                if first:
                    return tw[0:6, m * 128:(m + 1) * 128]
                return tw[:, k, m * 128:(m + 1) * 128]

            def rhs_h(k, u):
                if first:
                    return h_in[0:6, u:u + NT]
                return h_in[:, k, u:u + NT]

            scp = SCP.tile([128, NT], F32, tag="scp", name=f"scp{li}")
            for it in range(CCH // NT):
                u = it * NT
                psP = PP.tile([128, 2, NT], F32, tag="pp", name=f"psP{li}")
                for m in range(2):
                    for k in range(kt):
                        nc.tensor.matmul(psP[:, m, :], lhs_w(k, m), rhs_h(k, u),
                                         start=(k == 0), stop=(k == kt - 1))
                if not last:
                    psS = PSK.tile([128, 2, NT], F32, tag="psk", name=f"psS{li}")
                    for m in range(2):
                        for k in range(kt):
                            nc.tensor.matmul(psS[:, m, :], lhs_w(k, m + 2), rhs_h(k, u),
                                             start=(k == 0), stop=(k == kt - 1))
                for k in range(kt):
                    nc.tensor.matmul(scp[32 * it:32 * it + SR, :],
                                     (tab[0:6, :] if first else tab[:, k, :]),
                                     rhs_h(k, u), start=(k == 0), stop=(k == kt - 1),
                                     tile_position=(0, 32 * it))
                nc.scalar.activation(projc[:, :, u:u + NT], psP[:], ACTF.Copy)
                if not last:
                    for m in range(2):
                        nc.scalar.activation(acc[:, m, u:u + NT], psS[:, m, :],
                                             ACTF.Identity, bias=tb[:, m, :])
            nc.scalar.activation(scT[:], scp[:], ACTF.Copy)

            # scores -> DRAM -> (poly, head)-packed rows (head-major partitions)
            sc_dram = DR.tile([8, CCH], BF16, tag="sc_dram")
            scd = sc_dram[0:SR, :].rearrange("r (s w) -> r s w", w=NT)
            for s in range(4):
                nc.gpsimd.dma_start(scd[:, s, :], scT[32 * s:32 * s + SR, :])
            SP_ = PCH * Rl
            S = SM.tile([SP_, 2, V], BF16, tag="S")
            scv = sc_dram[0:SR, :].rearrange("(t r) (p v) -> r p t v", t=2, v=V)
            if last:
                nc.gpsimd.dma_start(S[:], scv[0])
            else:
                for r in range(R):
                    nc.gpsimd.dma_start(S[PCH * r:PCH * r + PCH], scv[r])
            if first:
                TT(S[:], S[:], s0a[:, :, ch:ch + 1].to_broadcast((128, 2, V)), op=ALU.add)

            # softmax over the 3 cycle edges, per (poly, head) row
            E = SM.tile([SP_, 3, V], BF16, tag="E")
            Ssrc, Stgt = S[:, 0, :], S[:, 1, :]
            TT(E[:, 0, 1:], Ssrc[:, :V - 1], Stgt[:, 1:], op=ALU.add)
            TT(E[:, 0, 0:1], Ssrc[:, V - 1:], Stgt[:, 0:1], op=ALU.add)
            TT(E[:, 1, :], Ssrc, Stgt, op=ALU.add)
            TT(E[:, 2, :V - 1], Ssrc[:, 1:], Stgt[:, :V - 1], op=ALU.add)
            TT(E[:, 2, V - 1:], Ssrc[:, 0:1], Stgt[:, V - 1:], op=ALU.add)
            STT(E[:], E[:], 0.2, E[:], op0=ALU.mult, op1=ALU.max)
            EX = SM.tile([SP_, 3, V], BF16, tag="EX")
            nc.scalar.activation(EX[:], E[:], ACTF.Exp)
            den = SM.tile([SP_, V], F32, tag="den")
            TT(den[:], EX[:, 0, :], EX[:, 1, :], op=ALU.add)
            TT(den[:], den[:], EX[:, 2, :], op=ALU.add)
            rd = SM.tile([SP_, V], F32, tag="rd")
            nc.vector.reciprocal(rd[:], den[:])
            ab = SM.tile([SP_, 3, V], BF16, tag="ab")
            TT(ab[:], EX[:], rd[:].unsqueeze(1).to_broadcast((SP_, 3, V)), op=ALU.mult)

            # alphas -> DRAM rows (h k t) -> af3 feature-row broadcast (per sub)
            afs = []
            if last:
                a_dram = DR.tile([3, CCH], BF16, tag="a_dram", name="a_dram3")
                nc.sync.dma_start(
                    a_dram[:].rearrange("k (p v) -> p k v", v=V), ab[:])
                for sc in range(CCH // SUB):
                    af3 = CH.tile([128, 3, SUB], BF16, tag="af3", name="af3l")
                    nc.sync.dma_start(
                        af3[:], a_dram[:, sc * SUB:(sc + 1) * SUB]
                        .unsqueeze(0).to_broadcast((128, 3, SUB)))
                    afs.append(af3)
            else:
                a_dram = DR.tile([12, CCH], BF16, tag="a_dram", name="a_dram12")
                adv = a_dram[:].rearrange("(h k t) (p v) -> t h p k v", h=2, k=3, v=V)
                for r in range(R):
                    tt_, hh = r // 2, r % 2
                    nc.gpsimd.dma_start(adv[tt_, hh], ab[PCH * r:PCH * r + PCH])
                a6 = a_dram[:].rearrange("(h kt) n -> h kt n", h=2)
                for sc in range(CCH // SUB):
                    af3 = CH.tile([128, 6, SUB], BF16, tag="af3", name="af3f")
                    for hh in range(2):
                        nc.sync.dma_start(
                            af3[64 * hh:64 * hh + 64, :, :],
                            a6[hh, :, sc * SUB:(sc + 1) * SUB]
                            .unsqueeze(0).to_broadcast((64, 6, SUB)))
                    afs.append(af3)
            return (li, ch, h_in, projc, acc, afs)

        def stageB(st, h_out):
            """Attention combine + residual/activation, writes h_out."""
            li, ch, h_in, projc, acc, afs = st
            first = li == 0
            last = li == 3
            tb = t_b[li]
            GV = SUB // V
            for sc in range(CCH // SUB):
                u = sc * SUB
                af3 = afs[sc]
                pj = projc[:, :, u:u + SUB].rearrange("f t (g v) -> f t g v", v=V)
                pjf = projc[:, :, u:u + SUB]
                av = acc[:, :, u:u + SUB]
                C = CH.tile([128, 2, SUB], BF16, tag="C")
                c = C[:].rearrange("f t (g v) -> f t g v", v=V)

                if last:
                    def afk(k):
                        return af3[:, k, :].rearrange("f (g v) -> f g v", v=V) \
                            .unsqueeze(1).to_broadcast((128, 2, GV, V))

                    def afkf(k):
                        return af3[:, k, :].unsqueeze(1).to_broadcast((128, 2, SUB))
                else:
                    def afk(k):
                        return af3[:, 2 * k:2 * k + 2, :].rearrange(
                            "f t (g v) -> f t g v", v=V)

                    def afkf(k):
                        return af3[:, 2 * k:2 * k + 2, :]

                if last:
                    TT(av, afkf(1), pjf, op=ALU.mult)       # acc = self term
                a2 = afk(2)
                TT(c[:, :, :, :V - 1], a2[:, :, :, :V - 1], pj[:, :, :, 1:], op=ALU.mult)
                TT(c[:, :, :, V - 1:], a2[:, :, :, V - 1:], pj[:, :, :, 0:1], op=ALU.mult)
                TT(av, av, C[:], op=ALU.add)
                a0 = afk(0)
                TT(c[:, :, :, 1:], a0[:, :, :, 1:], pj[:, :, :, :V - 1], op=ALU.mult)
                TT(c[:, :, :, 0:1], a0[:, :, :, 0:1], pj[:, :, :, V - 1:], op=ALU.mult)
                TT(av, av, C[:], op=ALU.add)
                if last:
                    TT(av, av, h_in[:, :, u:u + SUB], op=ALU.add)
                    for m in range(2):
                        nc.scalar.activation(h_out[:, m, u:u + SUB], acc[:, m, u:u + SUB],
                                             ACTF.Identity, bias=tb[:, m, :])
                    continue
                TT(C[:], afkf(1), pjf, op=ALU.mult)
                TT(av, av, C[:], op=ALU.add)
                if first:
                    g0 = ch * PCH + sc * (SUB // V)
                    gb = G0b[:, :, g0:g0 + GV].unsqueeze(3).to_broadcast(
                        (128, 2, GV, V))
                    avv = av.rearrange("f t (g v) -> f t g v", v=V)
                    TT(avv, avv, gb, op=ALU.add)
                # ELU: h = max(acc, min(exp(acc)-1, 0))
                nc.scalar.activation(C[:], av, ACTF.Exp)
                TSP(C[:], C[:], -1.0, 0.0, op0=ALU.add, op1=ALU.min)
                TT(h_out[:, :, u:u + SUB], C[:], av, op=ALU.max)

        def head(ch, hfin):
            u0 = ch * CCH
            yst = CH.tile([2, CCH], F32, tag="yst")
            for it in range(CCH // NT):
                u = it * NT
                psH = PP.tile([128, 2, NT], F32, tag="pp", name="psH")
                for m in range(2):
                    for k in range(2):
                        nc.tensor.matmul(psH[:, m, :], t_h1W[:, k, m * 128:(m + 1) * 128],
                                         hfin[:, k, u:u + NT], start=(k == 0), stop=(k == 1))
                h5 = SM.tile([128, 2, NT], BF16, tag="h5")
                for m in range(2):
                    nc.scalar.activation(h5[:, m, :], psH[:, m, :], ACTF.Silu,
                                         bias=t_h1b[:, m, :])
                ps2 = SCP.tile([2, NT], F32, tag="scp", name="ps2")
                for k in range(2):
                    nc.tensor.matmul(ps2[:], t_h2W[:, k, :], h5[:, k, :],
                                     start=(k == 0), stop=(k == 1))
                TSP(yst[:, u:u + NT], ps2[:], t_h2b[:], None, op0=ALU.add)
            nc.sync.dma_start(yT[:, u0:u0 + CCH], yst[:])

        for wv in range(NCH // WAVE):
            chs = list(range(wv * WAVE, (wv + 1) * WAVE))
            h_cur = {ch: None for ch in chs}
            for li in range(4):
                pend = None
                h_new = {}
                for ch in chs:
                    st = stageA(li, ch, h_cur[ch])
                    if pend is not None:
                        stageB(*pend)
                        if li == 3:
                            head(pend[0][1], pend[1])
                    h_new[ch] = HP.tile([128, 2, CCH], BF16, tag="h", name=f"h{li}_{ch}")
                    pend = (st, h_new[ch])
                stageB(*pend)
                if li == 3:
                    head(pend[0][1], pend[1])
                h_cur = h_new

    nc.compile()
    return nc


def kernel(**inputs):
    x = np.asarray(inputs["x"], np.float32)
    t = np.asarray(inputs["t"])
    nc = build(inputs)
    ph = np.arange(V, dtype=np.float32) * (2 * np.pi / V)
    posT = np.tile(np.stack([np.sin(ph), np.cos(ph), np.sin(2 * ph), np.cos(2 * ph)]),
                   (1, BC))
    in_maps = []
    for c in range(NCORES):
        xs = x[c * BC:(c + 1) * BC]
        xTs = np.ascontiguousarray(xs.reshape(N, 2).T)
        h6s = np.concatenate([xTs, posT], 0).astype(ml_dtypes.bfloat16)
        ts = t[c * BC:(c + 1) * BC].astype(np.float32)
        tps = np.ascontiguousarray(np.stack([ts, np.ones_like(ts)]))
        in_maps.append({"h6": np.ascontiguousarray(h6s), "tp": tps})
    res = run_bass_kernel_spmd(nc, in_maps, core_ids=list(range(NCORES)))
    outs = []
    for c in range(NCORES):
        yTs = res.results[c]["yT"]
        outs.append(yTs.T.reshape(BC, 2 * V).astype(np.float32))
    return np.concatenate(outs, 0)


# revision 22
# speedup vs baseline: 1.0848x; 1.0848x over previous
"""DenoiseGAT Trainium2 kernel: 8-core data-parallel over polygons (cycle graphs).

v2.1: layer-major software pipeline. Chunks of 2048 nodes (32 polygons)
stream through each GAT layer in waves of 4; per layer-phase, chunk c's
attention combine is emitted while chunk c+1's matmul/softmax stage runs,
so every engine sequencer sees a dense in-order stream. Activations h^T
(features x nodes, bf16) in [128, 2, n] half tiles. Scores come straight
from h via host-folded W@a_blk; softmax runs in a (poly, head)-packed
partition layout [128, 3, 64]; alphas return to feature rows via a
DRAM-staged broadcast DMA pair; the neighbor combine is +-1 shifted DVE
tensor ops inside 64-node polygons.
"""

import numpy as np
import ml_dtypes
from contextlib import ExitStack

import concourse.bass as bass
import concourse.tile as tile
import concourse.tile_utils as tile_utils
from concourse import bacc, mybir
from concourse.bass_utils import run_bass_kernel_spmd

tile_utils.max_sbuf_usage = 208 * 1024

F32 = mybir.dt.float32
BF16 = mybir.dt.bfloat16
ALU = mybir.AluOpType
ACTF = mybir.ActivationFunctionType

NCORES = 8
B, V = 2048, 64
HID, TDIM = 256, 128
R = 4                       # heads in layers 0-2
BC = B // NCORES            # 256 polygons / core
N = BC * V                  # 16384 nodes / core
CCH = 2048                  # chunk nodes = 32 polys
PCH = CCH // V              # 32 polys / chunk
NT = 512                    # matmul span (one PSUM bank col-width)
NCH = N // CCH              # 8 chunks
SUB = 1024                  # combine sub-chunk
WAVE = 4                    # chunks per wave


def _ablk(asrc, atgt):
    NH, FO = asrc.shape
    out = np.zeros((NH * FO, 2 * NH), np.float32)
    for h in range(NH):
        out[h * FO:(h + 1) * FO, h] = asrc[h]
        out[h * FO:(h + 1) * FO, NH + h] = atgt[h]
    return out


def _bf(a):
    return np.ascontiguousarray(np.asarray(a, np.float32).astype(ml_dtypes.bfloat16))


def _f32(a):
    return np.ascontiguousarray(np.asarray(a, np.float32))


def half3(a):
    """(256, X) host -> (128, 2, X) so tile[:, j, :] == rows 128j:128j+128."""
    a = np.asarray(a)
    return np.ascontiguousarray(a.reshape(2, 128, a.shape[1]).transpose(1, 0, 2))


def build(weights):
    nc = bacc.Bacc("TRN2", target_bir_lowering=False, debug=False,
                   enable_asserts=False, num_devices=NCORES)
    w = weights

    def inl(name, arr):
        return nc.inline_tensor(np.ascontiguousarray(arr), name=name).ap()

    half = TDIM // 2
    freqs = np.exp(-np.log(10000.0) * np.arange(half, dtype=np.float32) / (half - 1))
    fr2 = np.stack([np.concatenate([freqs, freqs]),
                    np.concatenate([np.zeros(half, np.float32),
                                    np.full(half, np.pi / 2, np.float32)])])

    W0 = _f32(w["W0"]); sk0 = _f32(w["skip0"]); ab0 = _ablk(_f32(w["asrc0"]), _f32(w["atgt0"]))
    c_fr2 = inl("fr2", fr2.astype(np.float32))
    c_tW = inl("tW", _f32(w["tW"]))
    c_tb = inl("tb", _f32(w["tb"]).reshape(-1, 1))
    c_Wsum0t = inl("Wsum0t", W0[6:] + sk0[6:])            # (128, 256) f32
    c_W0abt = inl("W0abt", W0[6:] @ ab0)                  # (128, 8) f32
    c_W0f = inl("W0f", _bf(np.concatenate([W0[:6], sk0[:6]], 1)))   # (6, 512)
    c_W0ab6 = inl("W0ab6", _bf(W0[:6] @ ab0))             # (6, 8)
    c_b0 = inl("b0c", half3(_f32(w["b0"]).reshape(-1, 1)))
    LW, LAB, LB = {}, {}, {}
    for i in (1, 2):
        Wi = _f32(w[f"W{i}"])
        abi = _ablk(_f32(w[f"asrc{i}"]), _f32(w[f"atgt{i}"]))
        LW[i] = inl(f"W{i}f", half3(_bf(np.concatenate([Wi, _f32(w[f"skip{i}"])], 1))))
        LAB[i] = inl(f"ab{i}f", half3(_bf(Wi @ abi)))     # (128, 2, 8)
        LB[i] = inl(f"b{i}c", half3(_f32(w[f"b{i}"]).reshape(-1, 1)))
    W3 = _f32(w["W3"]); ab3 = _ablk(_f32(w["asrc3"]), _f32(w["atgt3"]))
    c_W3 = inl("W3f", half3(_bf(W3)))
    c_Wab3 = inl("Wab3", half3(_bf(W3 @ ab3)))            # (128, 2, 2)
    c_b3 = inl("b3c", half3(_f32(w["b3"]).reshape(-1, 1)))
    c_h1W = inl("h1Wf", half3(_bf(_f32(w["h1W"]))))
    c_h1b = inl("h1bc", half3(_f32(w["h1b"]).reshape(-1, 1)))
    c_h2W = inl("h2Wf", half3(_bf(_f32(w["h2W"]))))
    c_h2b = inl("h2bc", _f32(w["h2b"]).reshape(-1, 1))

    h6 = nc.dram_tensor("h6", [6, N], BF16, kind="ExternalInput").ap()
    tp = nc.dram_tensor("tp", [2, BC], F32, kind="ExternalInput").ap()
    yT = nc.dram_tensor("yT", [2, N], F32, kind="ExternalOutput").ap()

    with tile.TileContext(nc) as tc, ExitStack() as ctx:
        WP = ctx.enter_context(tc.tile_pool(name="wts", bufs=1))
        P = ctx.enter_context(tc.tile_pool(name="pers", bufs=1))
        HP = ctx.enter_context(tc.tile_pool(name="hp", bufs=10))
        CH = ctx.enter_context(tc.tile_pool(name="ch", bufs=2))
        SM = ctx.enter_context(tc.tile_pool(name="sm", bufs=3))
        DR = ctx.enter_context(tc.tile_pool(name="dram", bufs=8, space="DRAM"))
        DR1 = ctx.enter_context(tc.tile_pool(name="dram1", bufs=1, space="DRAM"))
        PP = ctx.enter_context(tc.tile_pool(name="pp", bufs=2, space="PSUM"))
        PSK = ctx.enter_context(tc.tile_pool(name="psk", bufs=1, space="PSUM"))
        SCP = ctx.enter_context(tc.tile_pool(name="scp", bufs=2, space="PSUM"))

        TT = nc.vector.tensor_tensor
        PT = nc.gpsimd.tensor_tensor
        STT = nc.vector.scalar_tensor_tensor
        TSP = nc.vector.tensor_scalar

        def load(c_ap, tag):
            t = WP.tile(list(c_ap.shape), c_ap.dtype, tag=tag)
            nc.sync.dma_start(t[:], c_ap)
            return t

        # ---- temb preamble (once) ----
        t_fr2 = load(c_fr2, "fr2")
        t_tp = load(tp, "tp")
        ps_te = PP.tile([TDIM, BC], F32, tag="pp", name="ps_te")
        nc.tensor.matmul(ps_te[:], t_fr2[:], t_tp[:], start=True, stop=True)
        te_m = SM.tile([TDIM, BC], F32, tag="te", name="te_m")
        te_q = SM.tile([TDIM, BC], mybir.dt.int32, tag="teq")
        TSP(te_q[:], ps_te[:], float(1.0 / (2 * np.pi)), None, op0=ALU.mult)
        te_qf = SM.tile([TDIM, BC], F32, tag="te", name="te_qf")
        nc.vector.tensor_copy(te_qf[:], te_q[:])
        STT(te_m[:], te_qf[:], float(-2 * np.pi), ps_te[:], op0=ALU.mult, op1=ALU.add)
        te_s = SM.tile([TDIM, BC], F32, tag="te", name="te_s")
        nc.scalar.activation(te_s[:], te_m[:], ACTF.Sin)
        t_tW = load(c_tW, "tW")
        t_tb = load(c_tb, "tb")
        ps_tm = PP.tile([TDIM, BC], F32, tag="pp", name="ps_tm")
        nc.tensor.matmul(ps_tm[:], t_tW[:], te_s[:], start=True, stop=True)
        tembT = P.tile([TDIM, BC], F32, tag="tembT")
        nc.scalar.activation(tembT[:], ps_tm[:], ACTF.Silu, bias=t_tb[:])

        # G0 = (W0[6:]+skip0[6:]) @ temb : per-graph proj+skip contribution
        t_Ws0 = load(c_Wsum0t, "Ws0")
        G0b = P.tile([128, 2, BC], BF16, tag="G0b")
        for m in range(2):
            ps_g = PP.tile([128, BC], F32, tag="pp", name=f"ps_g{m}")
            nc.tensor.matmul(ps_g[:], t_Ws0[:, m * 128:(m + 1) * 128], tembT[:],
                             start=True, stop=True)
            nc.scalar.activation(G0b[:, m, :], ps_g[:], ACTF.Copy)
        # s0 = (W0[6:]@ab0)^T temb : per-graph score additive, (poly,head)-packed
        t_W0abt = load(c_W0abt, "W0abt")
        ps_sg = SCP.tile([8, BC], F32, tag="scp", name="ps_sg")
        nc.tensor.matmul(ps_sg[:], t_W0abt[:], tembT[:], start=True, stop=True)
        s0s = SM.tile([8, BC], BF16, tag="s0s")
        nc.scalar.activation(s0s[:], ps_sg[:], ACTF.Copy)
        s0_dram = DR1.tile([8, BC], BF16, tag="s0d")
        nc.sync.dma_start(s0_dram[:], s0s[:])
        s0a = P.tile([128, 2, NCH], BF16, tag="s0a")
        s0v = s0_dram[:].rearrange("(t r) (c p) -> r t p c", t=2, p=PCH)
        for r in range(R):
            for tt_ in range(2):
                nc.sync.dma_start(s0a[PCH * r:PCH * r + PCH, tt_, :], s0v[r, tt_])

        # ---- persistent weights ----
        t_W = {0: load(c_W0f, "W0f"), 1: load(LW[1], "W1"),
               2: load(LW[2], "W2"), 3: load(c_W3, "W3")}
        t_ab = {0: load(c_W0ab6, "W0ab6"), 1: load(LAB[1], "ab1"),
                2: load(LAB[2], "ab2"), 3: load(c_Wab3, "ab3")}
        t_b = {0: load(c_b0, "b0"), 1: load(LB[1], "b1"),
               2: load(LB[2], "b2"), 3: load(c_b3, "b3")}
        t_h1W = load(c_h1W, "h1W")
        t_h1b = load(c_h1b, "h1b")
        t_h2W = load(c_h2W, "h2W")
        t_h2b = load(c_h2b, "h2b")

        def stageA(li, ch, h_in):
            """Matmuls + evac + scores + softmax + alpha DMAs for one chunk.
            Returns state for stageB (the combine)."""
            first = li == 0
            last = li == 3
            Rl = 1 if last else R
            SR = 2 * Rl
            kt = 1 if first else 2
            tw, tab, tb = t_W[li], t_ab[li], t_b[li]

            if first:
                h_in = CH.tile([6, CCH], BF16, tag="h0")
                nc.sync.dma_start(h_in[:], h6[:, ch * CCH:(ch + 1) * CCH])
            projc = CH.tile([128, 2, CCH], BF16, tag="projc")
            acc = CH.tile([128, 2, CCH], BF16, tag="acc")
            scT = SM.tile([128, NT], BF16, tag="scT")

            def lhs_w(k, m):
                if first:
                    return tw[0:6, m * 128:(m + 1) * 128]
                return tw[:, k, m * 128:(m + 1) * 128]

            def rhs_h(k, u):
                if first:
                    return h_in[0:6, u:u + NT]
                return h_in[:, k, u:u + NT]

            scp = SCP.tile([128, NT], F32, tag="scp", name=f"scp{li}")
            for it in range(CCH // NT):
                u = it * NT
                psP = PP.tile([128, 2, NT], F32, tag="pp", name=f"psP{li}")
                for m in range(2):
                    for k in range(kt):
                        nc.tensor.matmul(psP[:, m, :], lhs_w(k, m), rhs_h(k, u),
                                         start=(k == 0), stop=(k == kt - 1))
                if not last:
                    psS = PSK.tile([128, 2, NT], F32, tag="psk", name=f"psS{li}")
                    for m in range(2):
                        for k in range(kt):
                            nc.tensor.matmul(psS[:, m, :], lhs_w(k, m + 2), rhs_h(k, u),
                                             start=(k == 0), stop=(k == kt - 1))
                for k in range(kt):
                    nc.tensor.matmul(scp[32 * it:32 * it + SR, :],
                                     (tab[0:6, :] if first else tab[:, k, :]),
                                     rhs_h(k, u), start=(k == 0), stop=(k == kt - 1),
                                     tile_position=(0, 32 * it))
                nc.scalar.activation(projc[:, :, u:u + NT], psP[:], ACTF.Copy)
                if not last:
                    for m in range(2):
                        nc.scalar.activation(acc[:, m, u:u + NT], psS[:, m, :],
                                             ACTF.Identity, bias=tb[:, m, :])
            nc.scalar.activation(scT[:], scp[:], ACTF.Copy)

            # scores -> DRAM -> (poly, head)-packed rows (head-major partitions)
            sc_dram = DR.tile([8, CCH], BF16, tag="sc_dram")
            scd = sc_dram[0:SR, :].rearrange("r (s w) -> r s w", w=NT)
            for s in range(4):
                nc.sync.dma_start(scd[:, s, :], scT[32 * s:32 * s + SR, :])
            SP_ = PCH * Rl
            S = SM.tile([SP_, 2, V], BF16, tag="S")
            scv = sc_dram[0:SR, :].rearrange("(t r) (p v) -> r p t v", t=2, v=V)
            if last:
                nc.gpsimd.dma_start(S[:], scv[0])
            else:
                for r in range(R):
                    nc.gpsimd.dma_start(S[PCH * r:PCH * r + PCH], scv[r])
            if first:
                TT(S[:], S[:], s0a[:, :, ch:ch + 1].to_broadcast((128, 2, V)), op=ALU.add)

            # softmax over the 3 cycle edges, per (poly, head) row
            E = SM.tile([SP_, 3, V], BF16, tag="E")
            Ssrc, Stgt = S[:, 0, :], S[:, 1, :]
            TT(E[:, 0, 1:], Ssrc[:, :V - 1], Stgt[:, 1:], op=ALU.add)
            TT(E[:, 0, 0:1], Ssrc[:, V - 1:], Stgt[:, 0:1], op=ALU.add)
            TT(E[:, 1, :], Ssrc, Stgt, op=ALU.add)
            TT(E[:, 2, :V - 1], Ssrc[:, 1:], Stgt[:, :V - 1], op=ALU.add)
            TT(E[:, 2, V - 1:], Ssrc[:, 0:1], Stgt[:, V - 1:], op=ALU.add)
            STT(E[:], E[:], 0.2, E[:], op0=ALU.mult, op1=ALU.max)
            EX = SM.tile([SP_, 3, V], BF16, tag="EX")
            nc.scalar.activation(EX[:], E[:], ACTF.Exp)
            den = SM.tile([SP_, V], F32, tag="den")
            TT(den[:], EX[:, 0, :], EX[:, 1, :], op=ALU.add)
            TT(den[:], den[:], EX[:, 2, :], op=ALU.add)
            rd = SM.tile([SP_, V], F32, tag="rd")
            nc.vector.reciprocal(rd[:], den[:])
            ab = SM.tile([SP_, 3, V], BF16, tag="ab")
            TT(ab[:], EX[:], rd[:].unsqueeze(1).to_broadcast((SP_, 3, V)), op=ALU.mult)

            # alphas -> DRAM rows (h k t) -> af3 feature-row broadcast (per sub)
            afs = []
            if last:
                a_dram = DR.tile([3, CCH], BF16, tag="a_dram", name="a_dram3")
                nc.sync.dma_start(
                    a_dram[:].rearrange("k (p v) -> p k v", v=V), ab[:])
                for sc in range(CCH // SUB):
                    af3 = CH.tile([128, 3, SUB], BF16, tag="af3", name="af3l")
                    nc.sync.dma_start(
                        af3[:], a_dram[:, sc * SUB:(sc + 1) * SUB]
                        .unsqueeze(0).to_broadcast((128, 3, SUB)))
                    afs.append(af3)
            else:
                a_dram = DR.tile([12, CCH], BF16, tag="a_dram", name="a_dram12")
                adv = a_dram[:].rearrange("(h k t) (p v) -> t h p k v", h=2, k=3, v=V)
                for r in range(R):
                    tt_, hh = r // 2, r % 2
                    nc.gpsimd.dma_start(adv[tt_, hh], ab[PCH * r:PCH * r + PCH])
                a6 = a_dram[:].rearrange("(h kt) n -> h kt n", h=2)
                for sc in range(CCH // SUB):
                    af3 = CH.tile([128, 6, SUB], BF16, tag="af3", name="af3f")
                    for hh in range(2):
                        nc.sync.dma_start(
                            af3[64 * hh:64 * hh + 64, :, :],
                            a6[hh, :, sc * SUB:(sc + 1) * SUB]
                            .unsqueeze(0).to_broadcast((64, 6, SUB)))
                    afs.append(af3)
            return (li, ch, h_in, projc, acc, afs)

        def stageB(st, h_out):
            """Attention combine + residual/activation, writes h_out."""
            li, ch, h_in, projc, acc, afs = st
            first = li == 0
            last = li == 3
            tb = t_b[li]
            GV = SUB // V
            for sc in range(CCH // SUB):
                u = sc * SUB
                af3 = afs[sc]
                pj = projc[:, :, u:u + SUB].rearrange("f t (g v) -> f t g v", v=V)
                pjf = projc[:, :, u:u + SUB]
                av = acc[:, :, u:u + SUB]
                C = CH.tile([128, 2, SUB], BF16, tag="C")
                c = C[:].rearrange("f t (g v) -> f t g v", v=V)

                if last:
                    def afk(k):
                        return af3[:, k, :].rearrange("f (g v) -> f g v", v=V) \
                            .unsqueeze(1).to_broadcast((128, 2, GV, V))

                    def afkf(k):
                        return af3[:, k, :].unsqueeze(1).to_broadcast((128, 2, SUB))
                else:
                    def afk(k):
                        return af3[:, 2 * k:2 * k + 2, :].rearrange(
                            "f t (g v) -> f t g v", v=V)

                    def afkf(k):
                        return af3[:, 2 * k:2 * k + 2, :]

                if last:
                    TT(av, afkf(1), pjf, op=ALU.mult)       # acc = self term
                a2 = afk(2)
                TT(c[:, :, :, :V - 1], a2[:, :, :, :V - 1], pj[:, :, :, 1:], op=ALU.mult)
                TT(c[:, :, :, V - 1:], a2[:, :, :, V - 1:], pj[:, :, :, 0:1], op=ALU.mult)
                TT(av, av, C[:], op=ALU.add)
                a0 = afk(0)
                TT(c[:, :, :, 1:], a0[:, :, :, 1:], pj[:, :, :, :V - 1], op=ALU.mult)
                TT(c[:, :, :, 0:1], a0[:, :, :, 0:1], pj[:, :, :, V - 1:], op=ALU.mult)
                TT(av, av, C[:], op=ALU.add)
                if last:
                    TT(av, av, h_in[:, :, u:u + SUB], op=ALU.add)
                    for m in range(2):
                        nc.scalar.activation(h_out[:, m, u:u + SUB], acc[:, m, u:u + SUB],
                                             ACTF.Identity, bias=tb[:, m, :])
                    continue
                TT(C[:], afkf(1), pjf, op=ALU.mult)
                TT(av, av, C[:], op=ALU.add)
                if first:
                    g0 = ch * PCH + sc * (SUB // V)
                    gb = G0b[:, :, g0:g0 + GV].unsqueeze(3).to_broadcast(
                        (128, 2, GV, V))
                    avv = av.rearrange("f t (g v) -> f t g v", v=V)
                    TT(avv, avv, gb, op=ALU.add)
                # ELU: h = max(acc, min(exp(acc)-1, 0))
                nc.scalar.activation(C[:], av, ACTF.Exp)
                TSP(C[:], C[:], -1.0, 0.0, op0=ALU.add, op1=ALU.min)
                TT(h_out[:, :, u:u + SUB], C[:], av, op=ALU.max)

        def head(ch, hfin):
            u0 = ch * CCH
            yst = CH.tile([2, CCH], F32, tag="yst")
            for it in range(CCH // NT):
                u = it * NT
                psH = PP.tile([128, 2, NT], F32, tag="pp", name="psH")
                for m in range(2):
                    for k in range(2):
                        nc.tensor.matmul(psH[:, m, :], t_h1W[:, k, m * 128:(m + 1) * 128],
                                         hfin[:, k, u:u + NT], start=(k == 0), stop=(k == 1))
                h5 = SM.tile([128, 2, NT], BF16, tag="h5")
                for m in range(2):
                    nc.scalar.activation(h5[:, m, :], psH[:, m, :], ACTF.Silu,
                                         bias=t_h1b[:, m, :])
                ps2 = SCP.tile([2, NT], F32, tag="scp", name="ps2")
                for k in range(2):
                    nc.tensor.matmul(ps2[:], t_h2W[:, k, :], h5[:, k, :],
                                     start=(k == 0), stop=(k == 1))
                TSP(yst[:, u:u + NT], ps2[:], t_h2b[:], None, op0=ALU.add)
            nc.sync.dma_start(yT[:, u0:u0 + CCH], yst[:])

        for wv in range(NCH // WAVE):
            chs = list(range(wv * WAVE, (wv + 1) * WAVE))
            h_cur = {ch: None for ch in chs}
            for li in range(4):
                pend = None
                h_new = {}
                for ch in chs:
                    st = stageA(li, ch, h_cur[ch])
                    if pend is not None:
                        stageB(*pend)
                        if li == 3:
                            head(pend[0][1], pend[1])
                    h_new[ch] = HP.tile([128, 2, CCH], BF16, tag="h", name=f"h{li}_{ch}")
                    pend = (st, h_new[ch])
                stageB(*pend)
                if li == 3:
                    head(pend[0][1], pend[1])
                h_cur = h_new

    nc.compile()
    return nc


def kernel(**inputs):
    x = np.asarray(inputs["x"], np.float32)
    t = np.asarray(inputs["t"])
    nc = build(inputs)
    ph = np.arange(V, dtype=np.float32) * (2 * np.pi / V)
    posT = np.tile(np.stack([np.sin(ph), np.cos(ph), np.sin(2 * ph), np.cos(2 * ph)]),
                   (1, BC))
    in_maps = []
    for c in range(NCORES):
        xs = x[c * BC:(c + 1) * BC]
        xTs = np.ascontiguousarray(xs.reshape(N, 2).T)
        h6s = np.concatenate([xTs, posT], 0).astype(ml_dtypes.bfloat16)
        ts = t[c * BC:(c + 1) * BC].astype(np.float32)
        tps = np.ascontiguousarray(np.stack([ts, np.ones_like(ts)]))
        in_maps.append({"h6": np.ascontiguousarray(h6s), "tp": tps})
    res = run_bass_kernel_spmd(nc, in_maps, core_ids=list(range(NCORES)))
    outs = []
    for c in range(NCORES):
        yTs = res.results[c]["yT"]
        outs.append(yTs.T.reshape(BC, 2 * V).astype(np.float32))
    return np.concatenate(outs, 0)
